# revision 9
# baseline (speedup 1.0000x reference)
"""Gated self-attention kernel for Trainium2, distributed over 8 NeuronCores.

Problem: out[b,q,:] = (softmax_k(Q[b] @ K[b]^T) @ V[b]) * V[b,q,:]
with B=4, S=4096, D=128, fp32.

Sharding: 8 cores = 4 batches x 2 query-halves. Each core computes 2048
query rows of one batch against the batch's full K/V.

Per-core algorithm (layouts chosen so NO on-device transposes are needed):
  - kt [128,4096] f16 = K[b]^T, qt [128,2048] f16 = Q[b,half]^T (d on
    partitions); vaug [128, 32*129] bf16 = V key-blocks with a ones column
    appended, so the PV matmul also produces the softmax denominator;
    vg [128,2048] f32 = V gate rows, partition-major blocks.
  - S^T[k,q] = kt_j^T @ qt in PSUM (f16 matmul, f32 accum).
  - P^T = exp(S^T - 60) -> SBUF bf16. The exp work is SPLIT across two
    engines: ScalarE Exp for 48 of 64 tiles, and a 2-instruction DVE
    fast-exp for 16 tiles (Schraudolph int trick: i32 = rint(s*A + B),
    then one 8-stage custom DVE op applies a quadratic mantissa
    correction: out = relu(bitcast(i32)) * P(M), max rel err 3.5e-3 --
    below the bf16 storage noise). ScalarE at ~1.07us/tile is the
    kernel's critical engine; DVE runs its tiles concurrently, cutting
    the exp wall from ~70us to ~53us. DVE tiles use a private 1-bank
    PSUM slot so their latency never stalls ScalarE's s-slot pipeline.
  - O_aug[q,0:129] += P^T_block^T @ vaug_j (col 128 accumulates l free).
  - out = (O / l) * gate on VectorE, streamed to DRAM per 256 columns.
"""

import numpy as np
import ml_dtypes

import concourse.bass as bass
import concourse.bacc as bacc
import concourse.mybir as mybir
import concourse.tile as tile
from concourse.bass_utils import run_bass_kernel_spmd

# ---------------------------------------------------------------------------
# EXP_CORR_ANT: custom DVE op for the 2-instruction fast exp.
# I1 (stock tensor_scalar): i32 = int32(s * A + B)
# I2 (this op): out = relu(bitcast_f32(i32)) * ((C2*M + C1)*M + C3),
#               M = bitcast((i32 & 0x007fffff) | 0x3f800000)
# Together they reconstruct exp(s - 60) to 0.35% max rel error, with deep
# underflow mapping to exactly 0 via the relu.
# ---------------------------------------------------------------------------
import concourse.dve_ops as dve_ops
from concourse.dve_ops import DveOp
from concourse.dve_spec import (
    AluOp, Bin, C0, C1, C2, C3, One, Spec, Src0, lower, relu,
    _spill_c3_to_src1, _has_src1,
)
from concourse.dve_uop import DveOpSpec

EXP_A = float(np.float32(2.0**23 * np.log2(np.e)))
EXP_B = float(np.float32(2.0**23 * (127.0 - 60.0 * np.log2(np.e))))
# quadratic minimax fit of 2^(M-1)/M on [1,2), rel err <= 3.5e-3
B0 = 1.457028199101779
B1 = -0.6941830124052303
B2 = 0.23369906190716086
MANT_MASK = 0x007FFFFF

_mant = Bin(AluOp.BITWISE_AND, Src0, C0)
_M = Bin(AluOp.BITWISE_OR, _mant, One)
_body = _spill_c3_to_src1(relu(Src0) * ((C2 * _M + C1) * _M + C3))


def _exp_corr_ref(in0, in1, s0, s1, imm2):
    i = np.ascontiguousarray(in0, dtype=np.float32).view(np.int32)
    m = ((i & np.int32(MANT_MASK)) | np.int32(0x3F800000)).view(np.float32)
    poly = (np.float32(imm2) * m + np.float32(s1)) * m + np.asarray(
        in1, np.float32).reshape(-1, 1)
    return np.maximum(np.ascontiguousarray(in0, np.float32), 0) * poly


_SPEC = Spec(body=_body, reference=_exp_corr_ref)


def _register_exp_corr():
    if "EXP_CORR_ANT" in dve_ops._SUB_OPCODE_FOR_NAME:
        return next(op for op in dve_ops.OPS if op.name == "EXP_CORR_ANT")
    row = max(dve_ops._SUB_OPCODE_FOR_NAME.values()) + 1
    assert row < 0x20
    dve_ops._SUB_OPCODE_FOR_NAME["EXP_CORR_ANT"] = row
    shas = {}
    for ver in ("v3", "v4"):
        sc = DveOpSpec(name="EXP_CORR_ANT", opcode=row,
                       uops=lower(_SPEC, ver=ver), rd1_en=_has_src1(_SPEC))
        shas[ver] = sc.sha(ver)
    op = DveOp("EXP_CORR_ANT", _SPEC, subdim=False, uops_sha=shas)
    dve_ops.OPS.append(op)
    dve_ops.CUSTOM_DVE_SPECS[op.name] = op.spec
    return op


# ---------------------------------------------------------------------------

P = 128
B, S, D = 4, 4096, 128
NCORES = 8
SQ = S // 2            # queries per core
NJ = S // P            # 32 key blocks
QC = 1024              # query chunk (PSUM-sized)
NQC = SQ // QC         # 2
NT = QC // P           # 8 q-blocks per chunk
EXP_BIAS = -60.0       # softmax shift; exact-cancels in normalization

F32 = mybir.dt.float32
F16 = mybir.dt.float16
BF16 = mybir.dt.bfloat16
I32 = mybir.dt.int32

_PROGRAM = None
_EXECUTOR = None

# DVE fast-exp j-blocks: spread through the chunk, away from the last js
# so the DVE tail never outlives ScalarE.
_DVE_JS = frozenset((2, 5, 9, 12, 16, 19, 23, 26))


def _dve_tile(j):
    return j in _DVE_JS


def _emit(tc, o_out, qt_in, kt_in, vaug_in, vg_in):
    nc = tc.nc
    Exp = mybir.ActivationFunctionType.Exp
    mult = mybir.AluOpType.mult
    add = mybir.AluOpType.add
    OP = _register_exp_corr()

    import contextlib
    with contextlib.ExitStack() as ctx:
        big = ctx.enter_context(tc.tile_pool(name="big", bufs=1))
        pt_pool = ctx.enter_context(tc.tile_pool(name="pt", bufs=4))
        i32_pool = ctx.enter_context(tc.tile_pool(name="i32", bufs=3))
        out_pool = ctx.enter_context(tc.tile_pool(name="outsb", bufs=2))
        small = ctx.enter_context(tc.tile_pool(name="small", bufs=4))
        s_pool = ctx.enter_context(tc.tile_pool(name="spsum", bufs=2, space="PSUM"))
        sdve_pool = ctx.enter_context(tc.tile_pool(name="sdve", bufs=1, space="PSUM"))
        oa_pool = ctx.enter_context(tc.tile_pool(name="oapsum", bufs=3, space="PSUM"))

        kt_sb = big.tile([P, S], F16)
        qt_sb = big.tile([P, SQ], F16)
        vaug_sb = big.tile([P, NJ * (D + 1)], BF16)
        vg_sb = big.tile([P, SQ], F32)
        bias_sb = big.tile([P, 1], F32)
        mask_sb = big.tile([P, 1], I32)
        b0_sb = big.tile([P, 1], F32)
        # Split loads so early matmuls only wait on their own slice. The
        # per-queue DMA rate is modest, so the first-needed slices are cut
        # fine (32-128KB) across many queues; the bulk streams behind the
        # compute.
        def split_load(dst, srcp, cuts):
            for a, b in zip(cuts[:-1], cuts[1:]):
                nc.sync.dma_start(dst[:, a:b], srcp[:, a:b])
        split_load(kt_sb, kt_in, [0, 128, 384, 1024, 2048, 3072, 4096])
        split_load(qt_sb, qt_in, [0, 256, 512, 1024, 1536, 2048])
        waug = NJ * (D + 1)
        split_load(vaug_sb, vaug_in, [0, 129, 387, waug // 4, waug // 2,
                                      3 * waug // 4, waug])
        split_load(vg_sb, vg_in, [0, SQ // 2, SQ])
        nc.vector.memset(bias_sb[:], EXP_BIAS)
        nc.vector.memset(mask_sb[:], MANT_MASK)
        nc.vector.memset(b0_sb[:], B0)

        for qc in range(NQC):
            # 8 q-block accumulators [128 q, 128 d + 1 l], packed 3/3/2 into
            # three single-bank PSUM tiles (129*3 fp32 = 1548B <= 2048B).
            oa_tiles = [oa_pool.tile([P, 3 * (D + 1)], F32, tag="oa",
                                     name=f"oa{qc}_{i}")
                        for i in range(3)]

            def emit_pv(j, pt):
                # One accumulation group per PSUM bank: start=True zeroes the
                # whole bank's has_written bits, so only the first write to
                # each bank may set it; later positions overwrite-on-clear.
                for t in range(NT):
                    ti, pos = divmod(t, 3)
                    oa = oa_tiles[ti]
                    nc.tensor.matmul(
                        oa[:, pos * (D + 1):(pos + 1) * (D + 1)],
                        pt[:, t * P:(t + 1) * P],
                        vaug_sb[:, j * (D + 1):(j + 1) * (D + 1)],
                        start=(j == 0 and pos == 0),
                        stop=(j == NJ - 1 and t in (2, 5, 7)),
                    )

            # software-pipelined: PV for block j-1 is emitted after QK+exp of
            # block j so TensorE never queue-blocks on the exp of the same j.
            pending = None
            for j in range(NJ):
                pt = pt_pool.tile([P, QC], BF16)
                if _dve_tile(j):
                    # DVE fast-exp: private 1-bank PSUM slot; the j-1 PV
                    # matmuls fill the PE gap while I1 drains each half.
                    i32 = i32_pool.tile([P, QC], I32, tag="i")
                    for h in range(QC // 512):
                        sd = sdve_pool.tile([P, 512], F32, tag="sd")
                        nc.tensor.matmul(
                            sd[:],
                            kt_sb[:, j * P:(j + 1) * P],
                            qt_sb[:, qc * QC + h * 512: qc * QC + (h + 1) * 512],
                            start=True, stop=True,
                        )
                        nc.vector.tensor_scalar(i32[:, h * 512:(h + 1) * 512],
                                                sd[:], EXP_A, EXP_B,
                                                mult, add)
                    nc.vector._custom_dve(OP, out=pt[:],
                                          in0=i32[:].bitcast(F32),
                                          in1=b0_sb[:],
                                          s0=mask_sb[:].bitcast(F32),
                                          s1=B1, imm2=B2)
                else:
                    s_ps = s_pool.tile([P, QC], F32, tag="s")
                    w = 256 if (j == 0 and qc == 0) else 512
                    for h in range(QC // w):
                        nc.tensor.matmul(
                            s_ps[:, h * w:(h + 1) * w],
                            kt_sb[:, j * P:(j + 1) * P],
                            qt_sb[:, qc * QC + h * w: qc * QC + (h + 1) * w],
                            start=True, stop=True,
                        )
                    nc.scalar.activation(pt[:], s_ps[:], Exp, bias=bias_sb[:])
                if pending is not None:
                    emit_pv(*pending)
                pending = (j, pt)
            emit_pv(*pending)

            out_sb = out_pool.tile([P, QC], F32)
            for t in range(NT):
                ti, pos = divmod(t, 3)
                oa = oa_tiles[ti]
                o_blk = oa[:, pos * (D + 1): pos * (D + 1) + D]
                l_col = oa[:, pos * (D + 1) + D: (pos + 1) * (D + 1)]
                invl = small.tile([P, 1], F32)
                nc.vector.reciprocal(invl[:], l_col)
                g = qc * NT + t
                if t % 2 == 0:
                    nc.vector.scalar_tensor_tensor(
                        out_sb[:, t * P:(t + 1) * P],
                        o_blk, invl[:], vg_sb[:, g * P:(g + 1) * P],
                        mult, mult,
                    )
                else:
                    # odd blocks ride ScalarE (o/l, per-partition scale) +
                    # GPSIMD (gate) so the tail endgame runs ~2x-parallel
                    # to VectorE's even blocks.
                    tmp = small.tile([P, P], F32, name=f"eg{qc}_{t}")
                    nc.scalar.activation(tmp[:], o_blk,
                                         mybir.ActivationFunctionType.Copy,
                                         scale=invl[:])
                    nc.gpsimd.tensor_tensor(out_sb[:, t * P:(t + 1) * P],
                                            tmp[:], vg_sb[:, g * P:(g + 1) * P],
                                            mult)
                if t % 2 == 1:  # stream results out as they materialize
                    a = qc * QC + (t - 1) * P
                    b = qc * QC + (t + 1) * P
                    if t == NT - 1:
                        # last chunk: split by partition halves on two queues
                        # (64 descriptors each) to halve the final DMA tail
                        for (pa, pb) in ((0, 64), (64, 128)):
                            nc.sync.dma_start(o_out[pa:pb, a:b],
                                              out_sb[pa:pb, (t - 1) * P:(t + 1) * P])
                    else:
                        nc.sync.dma_start(o_out[:, a:b],
                                          out_sb[:, (t - 1) * P:(t + 1) * P])


def build_program():
    # Bacc (not plain Bass): its compile() runs generate_event_semaphores,
    # which splits multi-sem waits to satisfy the TRN2 1-wait-per-instruction
    # constraint that walrus enforces.
    nc = bacc.Bacc("TRN2", target_bir_lowering=False, debug=False,
                   num_devices=NCORES)
    qt_in = nc.dram_tensor("qt", [P, SQ], F16, kind="ExternalInput").ap()
    kt_in = nc.dram_tensor("kt", [P, S], F16, kind="ExternalInput").ap()
    vaug_in = nc.dram_tensor("vaug", [P, NJ * (D + 1)], BF16,
                             kind="ExternalInput").ap()
    vg_in = nc.dram_tensor("vg", [P, SQ], F32, kind="ExternalInput").ap()
    o_out = nc.dram_tensor("o", [P, SQ], F32, kind="ExternalOutput").ap()
    with tile.TileContext(nc) as tc:
        _emit(tc, o_out, qt_in, kt_in, vaug_in, vg_in)
    nc.compile()
    return nc


def _get_program():
    global _PROGRAM
    if _PROGRAM is None:
        _PROGRAM = build_program()
    return _PROGRAM


def prep_core_inputs(Q, K, V, core, _cache={}):
    """Host-side shard + layout for one core. Batch-level conversions are
    cached across the two cores sharing a batch."""
    key = (id(Q), id(K), id(V))
    if _cache.get("key") != key:
        _cache.clear()
        _cache["key"] = key
        _cache["QT"] = np.ascontiguousarray(
            Q.transpose(0, 2, 1)).astype(np.float16)
        _cache["KT"] = np.ascontiguousarray(
            K.transpose(0, 2, 1)).astype(np.float16)
        vaug = np.ones((B, P, NJ, D + 1), dtype=ml_dtypes.bfloat16)
        vaug[:, :, :, :D] = V.reshape(B, NJ, P, D).transpose(0, 2, 1, 3
                                                             ).astype(ml_dtypes.bfloat16)
        _cache["VAUG"] = np.ascontiguousarray(
            vaug.reshape(B, P, NJ * (D + 1)))
        _cache["VG"] = np.ascontiguousarray(
            V.reshape(B, NJ, P, D).transpose(0, 2, 1, 3).reshape(B, P, S)
        ).astype(np.float32)
    b, h = divmod(core, 2)
    qt = np.ascontiguousarray(_cache["QT"][b][:, h * SQ:(h + 1) * SQ])
    vg = np.ascontiguousarray(_cache["VG"][b][:, h * SQ:(h + 1) * SQ])
    return {"qt": qt, "kt": _cache["KT"][b], "vaug": _cache["VAUG"][b],
            "vg": vg}


def assemble_output(results):
    out = np.empty((B, S, D), dtype=np.float32)
    for core in range(NCORES):
        b, h = divmod(core, 2)
        o = results[core]["o"]  # [P, SQ]
        out[b, h * SQ:(h + 1) * SQ, :] = (
            o.reshape(P, SQ // P, D).transpose(1, 0, 2).reshape(SQ, D))
    return out


def _build_executor(nc):
    """Persistent jitted shard_map executor (run_bass_via_pjrt re-traces and
    re-jits on every call; this builds the dispatchable once)."""
    import jax
    from jax.sharding import Mesh, PartitionSpec
    from jax.experimental.shard_map import shard_map
    from concourse import bass2jax

    bass2jax.install_neuronx_cc_hook()
    pname = nc.partition_id_tensor.name if nc.partition_id_tensor else None
    in_names, out_names, out_avals, zero_outs = [], [], [], []
    for alloc in nc.m.functions[0].allocations:
        if not isinstance(alloc, mybir.MemoryLocationSet):
            continue
        name = alloc.memorylocations[0].name
        if alloc.kind == "ExternalInput":
            if name != pname:
                in_names.append(name)
        elif alloc.kind == "ExternalOutput":
            out_names.append(name)
            shape = tuple(alloc.tensor_shape)
            dtype = mybir.dt.np(alloc.dtype)
            out_avals.append(jax.core.ShapedArray(shape, dtype))
            zero_outs.append(np.zeros(shape, dtype))
    n_params = len(in_names)
    all_names = in_names + out_names + ([pname] if pname else [])

    def _body(*args):
        ops = list(args)
        if pname is not None:
            ops.append(bass2jax.partition_id_tensor())
        outs = bass2jax._bass_exec_p.bind(
            *ops, out_avals=tuple(out_avals), in_names=tuple(all_names),
            out_names=tuple(out_names), lowering_input_output_aliases=(),
            sim_require_finite=True, sim_require_nnan=True, nc=nc)
        return tuple(outs)

    devices = jax.devices()[:NCORES]
    mesh = Mesh(np.asarray(devices), ("core",))
    specs = (PartitionSpec("core"),) * (n_params + len(out_names))
    out_specs = (PartitionSpec("core"),) * len(out_names)
    fn = jax.jit(shard_map(_body, mesh=mesh, in_specs=specs,
                           out_specs=out_specs, check_rep=False),
                 keep_unused=True)
    concat_zero = [np.zeros((NCORES * z.shape[0],) + z.shape[1:], z.dtype)
                   for z in zero_outs]

    def run(in_maps):
        concat_in = [np.concatenate([in_maps[c][n] for c in range(NCORES)],
                                    axis=0) for n in in_names]
        outs = fn(*concat_in, *concat_zero)
        o_all = np.asarray(outs[0]).reshape(NCORES, P, SQ)
        return [{"o": o_all[c]} for c in range(NCORES)]

    return run


def kernel(Q, K, V):
    global _EXECUTOR
    Q = np.asarray(Q, dtype=np.float32)
    K = np.asarray(K, dtype=np.float32)
    V = np.asarray(V, dtype=np.float32)
    nc = _get_program()
    in_maps = [prep_core_inputs(Q, K, V, c) for c in range(NCORES)]
    try:
        if _EXECUTOR is None:
            _EXECUTOR = _build_executor(nc)
        res = _EXECUTOR(in_maps)
    except Exception:
        _EXECUTOR = None
        res = run_bass_kernel_spmd(nc, in_maps, list(range(NCORES))).results
    return assemble_output(res)


# revision 11
# speedup vs baseline: 1.1965x; 1.1965x over previous
"""Gated self-attention kernel for Trainium2, distributed over 8 NeuronCores.

Problem: out[b,q,:] = (softmax_k(Q[b] @ K[b]^T) @ V[b]) * V[b,q,:]
with B=4, S=4096, D=128, fp32.

Sharding: 8 cores = 4 batches x 2 query-halves. Each core computes 2048
query rows of one batch against the batch's full K/V.

Per-core algorithm (layouts chosen so NO on-device transposes are needed):
  - kt [128,4096] f16 = K[b]^T, qt [128,2048] f16 = Q[b,half]^T (d on
    partitions); vaug [128, 32*129] bf16 = V key-blocks with a ones column
    appended, so the PV matmul also produces the softmax denominator;
    vg [128,2048] f32 = V gate rows, partition-major blocks.
  - S^T[k,q] = kt_j^T @ qt in PSUM (f16 matmul, f32 accum).
  - P^T = exp(S^T - 60) -> SBUF bf16. The exp work is SPLIT across two
    engines: ScalarE Exp for 48 of 64 tiles, and a 2-instruction DVE
    fast-exp for 16 tiles (Schraudolph int trick: i32 = rint(s*A + B),
    then one 8-stage custom DVE op applies a quadratic mantissa
    correction: out = relu(bitcast(i32)) * P(M), max rel err 3.5e-3 --
    below the bf16 storage noise). ScalarE at ~1.07us/tile is the
    kernel's critical engine; DVE runs its tiles concurrently, cutting
    the exp wall from ~70us to ~53us. DVE tiles use a private 1-bank
    PSUM slot so their latency never stalls ScalarE's s-slot pipeline.
  - O_aug[q,0:129] += P^T_block^T @ vaug_j (col 128 accumulates l free).
  - out = (O / l) * gate on VectorE, streamed to DRAM per 256 columns.
"""

import numpy as np
import ml_dtypes

import concourse.bass as bass
import concourse.bacc as bacc
import concourse.mybir as mybir
import concourse.tile as tile
from concourse.bass_utils import run_bass_kernel_spmd

# ---------------------------------------------------------------------------
# EXP_CORR_ANT: custom DVE op for the 2-instruction fast exp.
# I1 (stock tensor_scalar): i32 = int32(s * A + B)
# I2 (this op): out = relu(bitcast_f32(i32)) * ((C2*M + C1)*M + C3),
#               M = bitcast((i32 & 0x007fffff) | 0x3f800000)
# Together they reconstruct exp(s - 60) to 0.35% max rel error, with deep
# underflow mapping to exactly 0 via the relu.
# ---------------------------------------------------------------------------
import concourse.dve_ops as dve_ops
from concourse.dve_ops import DveOp
from concourse.dve_spec import (
    AluOp, Bin, C0, C1, C2, C3, One, Spec, Src0, lower, relu,
    _spill_c3_to_src1, _has_src1,
)
from concourse.dve_uop import DveOpSpec

EXP_A = float(np.float32(2.0**23 * np.log2(np.e)))
EXP_B = float(np.float32(2.0**23 * (127.0 - 60.0 * np.log2(np.e))))
# quadratic minimax fit of 2^(M-1)/M on [1,2), rel err <= 3.5e-3
B0 = 1.457028199101779
B1 = -0.6941830124052303
B2 = 0.23369906190716086
MANT_MASK = 0x007FFFFF

_mant = Bin(AluOp.BITWISE_AND, Src0, C0)
_M = Bin(AluOp.BITWISE_OR, _mant, One)
_body = _spill_c3_to_src1(relu(Src0) * ((C2 * _M + C1) * _M + C3))


def _exp_corr_ref(in0, in1, s0, s1, imm2):
    i = np.ascontiguousarray(in0, dtype=np.float32).view(np.int32)
    m = ((i & np.int32(MANT_MASK)) | np.int32(0x3F800000)).view(np.float32)
    poly = (np.float32(imm2) * m + np.float32(s1)) * m + np.asarray(
        in1, np.float32).reshape(-1, 1)
    return np.maximum(np.ascontiguousarray(in0, np.float32), 0) * poly


_SPEC = Spec(body=_body, reference=_exp_corr_ref)


def _register_exp_corr():
    if "EXP_CORR_ANT" in dve_ops._SUB_OPCODE_FOR_NAME:
        return next(op for op in dve_ops.OPS if op.name == "EXP_CORR_ANT")
    row = max(dve_ops._SUB_OPCODE_FOR_NAME.values()) + 1
    assert row < 0x20
    dve_ops._SUB_OPCODE_FOR_NAME["EXP_CORR_ANT"] = row
    shas = {}
    for ver in ("v3", "v4"):
        sc = DveOpSpec(name="EXP_CORR_ANT", opcode=row,
                       uops=lower(_SPEC, ver=ver), rd1_en=_has_src1(_SPEC))
        shas[ver] = sc.sha(ver)
    op = DveOp("EXP_CORR_ANT", _SPEC, subdim=False, uops_sha=shas)
    dve_ops.OPS.append(op)
    dve_ops.CUSTOM_DVE_SPECS[op.name] = op.spec
    return op


# ---------------------------------------------------------------------------

P = 128
B, S, D = 4, 4096, 128
NCORES = 8
SQ = S // 2            # queries per core
NJ = S // P            # 32 key blocks
QC = 1024              # query chunk (PSUM-sized)
NQC = SQ // QC         # 2
NT = QC // P           # 8 q-blocks per chunk
EXP_BIAS = -60.0       # softmax shift; exact-cancels in normalization

F32 = mybir.dt.float32
F16 = mybir.dt.float16
BF16 = mybir.dt.bfloat16
I32 = mybir.dt.int32

_PROGRAM = None
_EXECUTOR = None

# DVE fast-exp j-blocks: spread through the chunk, away from the last js
# so the DVE tail never outlives ScalarE.
_DVE_JS = frozenset((2, 5, 9, 12, 16, 19, 23, 26))


def _dve_tile(j):
    return j in _DVE_JS


def _emit(tc, o_out, qt_in, kt_in, vaug_in, vg_in):
    nc = tc.nc
    Exp = mybir.ActivationFunctionType.Exp
    mult = mybir.AluOpType.mult
    add = mybir.AluOpType.add
    OP = _register_exp_corr()

    import contextlib
    with contextlib.ExitStack() as ctx:
        big = ctx.enter_context(tc.tile_pool(name="big", bufs=1))
        pt_pool = ctx.enter_context(tc.tile_pool(name="pt", bufs=4))
        i32_pool = ctx.enter_context(tc.tile_pool(name="i32", bufs=3))
        out_pool = ctx.enter_context(tc.tile_pool(name="outsb", bufs=2))
        small = ctx.enter_context(tc.tile_pool(name="small", bufs=4))
        s_pool = ctx.enter_context(tc.tile_pool(name="spsum", bufs=2, space="PSUM"))
        sdve_pool = ctx.enter_context(tc.tile_pool(name="sdve", bufs=1, space="PSUM"))
        oa_pool = ctx.enter_context(tc.tile_pool(name="oapsum", bufs=3, space="PSUM"))

        kt_sb = big.tile([P, S], F16)
        qt_sb = big.tile([P, SQ], F16)
        vaug_sb = big.tile([P, NJ * (D + 1)], BF16)
        vg_sb = big.tile([P, SQ], F32)
        bias_sb = big.tile([P, 1], F32)
        mask_sb = big.tile([P, 1], I32)
        b0_sb = big.tile([P, 1], F32)
        nc.vector.memset(bias_sb[:], EXP_BIAS)
        nc.vector.memset(mask_sb[:], MANT_MASK)
        nc.vector.memset(b0_sb[:], B0)
        # Split loads so early matmuls only wait on their own slice. The
        # per-queue DMA rate is modest, so the first-needed slices are cut
        # fine (32-128KB) across many queues; the bulk streams behind the
        # compute.
        def split_load(dst, srcp, cuts):
            for a, b in zip(cuts[:-1], cuts[1:]):
                nc.sync.dma_start(dst[:, a:b], srcp[:, a:b])
        split_load(kt_sb, kt_in, [0, 128, 384, 1024, 2048, 3072, 4096])
        split_load(qt_sb, qt_in, [0, 256, 512, 1024, 1536, 2048])
        waug = NJ * (D + 1)
        split_load(vaug_sb, vaug_in, [0, 129, 387, waug // 4, waug // 2,
                                      3 * waug // 4, waug])
        split_load(vg_sb, vg_in, [0, SQ // 2, SQ])
        # Warmup activation: the first Exp triggers walrus's ACT_TABLE_LOAD
        # insertion, which tolerates only a single sync-wait on that
        # instruction. Emitted after the input dma_starts; one dep (the
        # memset) keeps it off any critical sync chain, and the table is
        # loaded long before the first real exp needs it.
        warm_sb = big.tile([P, 1], F32)
        nc.scalar.activation(warm_sb[:], bias_sb[:],
                             mybir.ActivationFunctionType.Exp,
                             bias=bias_sb[:])

        for qc in range(NQC):
            # 8 q-block accumulators [128 q, 128 d + 1 l], packed 3/3/2 into
            # three single-bank PSUM tiles (129*3 fp32 = 1548B <= 2048B).
            oa_tiles = [oa_pool.tile([P, 3 * (D + 1)], F32, tag="oa",
                                     name=f"oa{qc}_{i}")
                        for i in range(3)]

            def emit_pv(j, pt):
                # One accumulation group per PSUM bank: start=True zeroes the
                # whole bank's has_written bits, so only the first write to
                # each bank may set it; later positions overwrite-on-clear.
                for t in range(NT):
                    ti, pos = divmod(t, 3)
                    oa = oa_tiles[ti]
                    nc.tensor.matmul(
                        oa[:, pos * (D + 1):(pos + 1) * (D + 1)],
                        pt[:, t * P:(t + 1) * P],
                        vaug_sb[:, j * (D + 1):(j + 1) * (D + 1)],
                        start=(j == 0 and pos == 0),
                        stop=(j == NJ - 1 and t in (2, 5, 7)),
                    )

            # software-pipelined: PV for block j-1 is emitted after QK+exp of
            # block j so TensorE never queue-blocks on the exp of the same j.
            pending = None
            for j in range(NJ):
                pt = pt_pool.tile([P, QC], BF16)
                if _dve_tile(j):
                    # DVE fast-exp: private 1-bank PSUM slot; the j-1 PV
                    # matmuls fill the PE gap while I1 drains each half.
                    i32 = i32_pool.tile([P, QC], I32, tag="i")
                    for h in range(QC // 512):
                        sd = sdve_pool.tile([P, 512], F32, tag="sd")
                        nc.tensor.matmul(
                            sd[:],
                            kt_sb[:, j * P:(j + 1) * P],
                            qt_sb[:, qc * QC + h * 512: qc * QC + (h + 1) * 512],
                            start=True, stop=True,
                        )
                        nc.vector.tensor_scalar(i32[:, h * 512:(h + 1) * 512],
                                                sd[:], EXP_A, EXP_B,
                                                mult, add)
                    nc.vector._custom_dve(OP, out=pt[:],
                                          in0=i32[:].bitcast(F32),
                                          in1=b0_sb[:],
                                          s0=mask_sb[:].bitcast(F32),
                                          s1=B1, imm2=B2)
                else:
                    s_ps = s_pool.tile([P, QC], F32, tag="s")
                    w = 256 if (j == 0 and qc == 0) else 512
                    for h in range(QC // w):
                        nc.tensor.matmul(
                            s_ps[:, h * w:(h + 1) * w],
                            kt_sb[:, j * P:(j + 1) * P],
                            qt_sb[:, qc * QC + h * w: qc * QC + (h + 1) * w],
                            start=True, stop=True,
                        )
                    nc.scalar.activation(pt[:], s_ps[:], Exp, bias=bias_sb[:])
                if pending is not None:
                    emit_pv(*pending)
                pending = (j, pt)
            emit_pv(*pending)

            out_sb = out_pool.tile([P, QC], F32)
            for t in range(NT):
                ti, pos = divmod(t, 3)
                oa = oa_tiles[ti]
                o_blk = oa[:, pos * (D + 1): pos * (D + 1) + D]
                l_col = oa[:, pos * (D + 1) + D: (pos + 1) * (D + 1)]
                invl = small.tile([P, 1], F32)
                nc.vector.reciprocal(invl[:], l_col)
                g = qc * NT + t
                if t % 2 == 0:
                    nc.vector.scalar_tensor_tensor(
                        out_sb[:, t * P:(t + 1) * P],
                        o_blk, invl[:], vg_sb[:, g * P:(g + 1) * P],
                        mult, mult,
                    )
                else:
                    # odd blocks ride ScalarE (o/l, per-partition scale) +
                    # GPSIMD (gate) so the tail endgame runs ~2x-parallel
                    # to VectorE's even blocks.
                    tmp = small.tile([P, P], F32, name=f"eg{qc}_{t}")
                    nc.scalar.activation(tmp[:], o_blk,
                                         mybir.ActivationFunctionType.Copy,
                                         scale=invl[:])
                    nc.gpsimd.tensor_tensor(out_sb[:, t * P:(t + 1) * P],
                                            tmp[:], vg_sb[:, g * P:(g + 1) * P],
                                            mult)
                if t % 2 == 1:  # stream results out as they materialize
                    a = qc * QC + (t - 1) * P
                    b = qc * QC + (t + 1) * P
                    if t == NT - 1:
                        # last chunk: partition-halved onto two queues (64
                        # descriptors each) to halve the final DMA tail;
                        # safe on the output side (no consumer sem chain)
                        for (pa, pb) in ((0, 64), (64, 128)):
                            nc.sync.dma_start(
                                o_out[pa:pb, a:b],
                                out_sb[pa:pb, (t - 1) * P:(t + 1) * P])
                    else:
                        nc.sync.dma_start(o_out[:, a:b],
                                          out_sb[:, (t - 1) * P:(t + 1) * P])


def build_program():
    # Bacc (not plain Bass): its compile() runs generate_event_semaphores,
    # which splits multi-sem waits to satisfy the TRN2 1-wait-per-instruction
    # constraint that walrus enforces.
    nc = bacc.Bacc("TRN2", target_bir_lowering=False, debug=False,
                   num_devices=NCORES)
    qt_in = nc.dram_tensor("qt", [P, SQ], F16, kind="ExternalInput").ap()
    kt_in = nc.dram_tensor("kt", [P, S], F16, kind="ExternalInput").ap()
    vaug_in = nc.dram_tensor("vaug", [P, NJ * (D + 1)], BF16,
                             kind="ExternalInput").ap()
    vg_in = nc.dram_tensor("vg", [P, SQ], F32, kind="ExternalInput").ap()
    o_out = nc.dram_tensor("o", [P, SQ], F32, kind="ExternalOutput").ap()
    with tile.TileContext(nc) as tc:
        _emit(tc, o_out, qt_in, kt_in, vaug_in, vg_in)
    nc.compile()
    return nc


def _get_program():
    global _PROGRAM
    if _PROGRAM is None:
        _PROGRAM = build_program()
    return _PROGRAM


def prep_core_inputs(Q, K, V, core, _cache={}):
    """Host-side shard + layout for one core. Batch-level conversions are
    cached across the two cores sharing a batch."""
    key = (id(Q), id(K), id(V))
    if _cache.get("key") != key:
        _cache.clear()
        _cache["key"] = key
        _cache["QT"] = np.ascontiguousarray(
            Q.transpose(0, 2, 1)).astype(np.float16)
        _cache["KT"] = np.ascontiguousarray(
            K.transpose(0, 2, 1)).astype(np.float16)
        vaug = np.ones((B, P, NJ, D + 1), dtype=ml_dtypes.bfloat16)
        vaug[:, :, :, :D] = V.reshape(B, NJ, P, D).transpose(0, 2, 1, 3
                                                             ).astype(ml_dtypes.bfloat16)
        _cache["VAUG"] = np.ascontiguousarray(
            vaug.reshape(B, P, NJ * (D + 1)))
        _cache["VG"] = np.ascontiguousarray(
            V.reshape(B, NJ, P, D).transpose(0, 2, 1, 3).reshape(B, P, S)
        ).astype(np.float32)
    b, h = divmod(core, 2)
    qt = np.ascontiguousarray(_cache["QT"][b][:, h * SQ:(h + 1) * SQ])
    vg = np.ascontiguousarray(_cache["VG"][b][:, h * SQ:(h + 1) * SQ])
    return {"qt": qt, "kt": _cache["KT"][b], "vaug": _cache["VAUG"][b],
            "vg": vg}


def assemble_output(results):
    out = np.empty((B, S, D), dtype=np.float32)
    for core in range(NCORES):
        b, h = divmod(core, 2)
        o = results[core]["o"]  # [P, SQ]
        out[b, h * SQ:(h + 1) * SQ, :] = (
            o.reshape(P, SQ // P, D).transpose(1, 0, 2).reshape(SQ, D))
    return out


def _build_executor(nc):
    """Persistent jitted shard_map executor (run_bass_via_pjrt re-traces and
    re-jits on every call; this builds the dispatchable once)."""
    import jax
    from jax.sharding import Mesh, PartitionSpec
    from jax.experimental.shard_map import shard_map
    from concourse import bass2jax

    bass2jax.install_neuronx_cc_hook()
    pname = nc.partition_id_tensor.name if nc.partition_id_tensor else None
    in_names, out_names, out_avals, zero_outs = [], [], [], []
    for alloc in nc.m.functions[0].allocations:
        if not isinstance(alloc, mybir.MemoryLocationSet):
            continue
        name = alloc.memorylocations[0].name
        if alloc.kind == "ExternalInput":
            if name != pname:
                in_names.append(name)
        elif alloc.kind == "ExternalOutput":
            out_names.append(name)
            shape = tuple(alloc.tensor_shape)
            dtype = mybir.dt.np(alloc.dtype)
            out_avals.append(jax.core.ShapedArray(shape, dtype))
            zero_outs.append(np.zeros(shape, dtype))
    n_params = len(in_names)
    all_names = in_names + out_names + ([pname] if pname else [])

    def _body(*args):
        ops = list(args)
        if pname is not None:
            ops.append(bass2jax.partition_id_tensor())
        outs = bass2jax._bass_exec_p.bind(
            *ops, out_avals=tuple(out_avals), in_names=tuple(all_names),
            out_names=tuple(out_names), lowering_input_output_aliases=(),
            sim_require_finite=True, sim_require_nnan=True, nc=nc)
        return tuple(outs)

    devices = jax.devices()[:NCORES]
    mesh = Mesh(np.asarray(devices), ("core",))
    specs = (PartitionSpec("core"),) * (n_params + len(out_names))
    out_specs = (PartitionSpec("core"),) * len(out_names)
    fn = jax.jit(shard_map(_body, mesh=mesh, in_specs=specs,
                           out_specs=out_specs, check_rep=False),
                 keep_unused=True)
    concat_zero = [np.zeros((NCORES * z.shape[0],) + z.shape[1:], z.dtype)
                   for z in zero_outs]

    def run(in_maps):
        concat_in = [np.concatenate([in_maps[c][n] for c in range(NCORES)],
                                    axis=0) for n in in_names]
        outs = fn(*concat_in, *concat_zero)
        o_all = np.asarray(outs[0]).reshape(NCORES, P, SQ)
        return [{"o": o_all[c]} for c in range(NCORES)]

    return run


def kernel(Q, K, V):
    global _EXECUTOR
    Q = np.asarray(Q, dtype=np.float32)
    K = np.asarray(K, dtype=np.float32)
    V = np.asarray(V, dtype=np.float32)
    nc = _get_program()
    in_maps = [prep_core_inputs(Q, K, V, c) for c in range(NCORES)]
    try:
        if _EXECUTOR is None:
            _EXECUTOR = _build_executor(nc)
        res = _EXECUTOR(in_maps)
    except Exception:
        _EXECUTOR = None
        res = run_bass_kernel_spmd(nc, in_maps, list(range(NCORES))).results
    return assemble_output(res)
